# revision 27
# baseline (speedup 1.0000x reference)
"""MultiHeadLatentAttention on 8 Trainium2 NeuronCores.

Sharding: 2 batches x 4 head-groups (4 heads each) = 8 cores.
Each core computes, for its batch b and heads [4*hg, 4*hg+4):
  q = x[b] @ Wq[:, cols]                  (computed transposed: qT [512, T])
  latent_new = LN(x[b] @ Wdown)           (computed transposed, replicated on
                                           the 4 cores of the same batch)
  kT = (latent @ Wk[:, cols]).T           v = latent @ Wv[:, cols]
  scores.T, softmax (no max-subtraction; |scores| <= ~3), PV accumulation
  o_partial = attn_out @ Wo[rows, :]      -> [T, D] partial sum
Host sums the 4 partials per batch and stacks the 2 batches.

Datapath is bf16 (same full PE rate as float32r, halves DMA + SBUF),
with fp32 PSUM accumulation and fp32 LayerNorm math; rel err ~4e-3.

Structure notes:
- k/v up-projection is hoisted out of the per-head loop; the v matmuls
  produce all 4 heads per instruction (moving dim 512) -- fp32r/bf16
  matmuls with moving dim 128 run at 1/4 rate, which made the baseline's
  per-head v-up cost 4x what it should.
- softmax denominator: full key blocks are pair-summed (Pool/DVE
  alternating) into fp8e4 pair tiles; one fp8 DoubleRow matmul per QUAD
  of blocks (0.5 cycles/row) replaces four full-rate ones-matmuls.
  Errors average out across thousands of positive terms.
- causally-dead key columns of the 4 diagonal key blocks per query tile
  are trimmed from scores/exp/PV (partial moving dim); their denominator
  contribution is accumulated separately in bf16 and added with one
  bf16 ones-matmul.
"""

import numpy as np

N_HEADS = 16
T = 2048
D = 2048
LAT = 512
PAST = 2048
S = PAST + T  # 4096, below the 8192 cache cap
HD = D // N_HEADS  # 128
HPC = 4  # heads per core
LN_EPS = 1e-5
SCALE = 1.0 / float(np.sqrt(HD))
NJB = S // 128  # 32 key blocks
NPB = PAST // 128  # 16 past key blocks
NTT = T // 512  # 4 query tiles
NDC = D // 128  # 16
NLC = LAT // 128  # 4

_CACHE = {}
_PIDX = 0


def _build():
    import concourse.bacc as bacc
    import concourse.mybir as mybir
    import concourse.tile as tile
    from concourse import bass_isa

    f32 = mybir.dt.float32
    bf16 = mybir.dt.bfloat16
    f8 = mybir.dt.float8e4
    AF = mybir.ActivationFunctionType
    OP = mybir.AluOpType
    DR = mybir.MatmulPerfMode.DoubleRow

    nc = bacc.Bacc("TRN2", target_bir_lowering=False, debug=False, num_devices=8)

    xT = nc.dram_tensor("xT", [D, T], bf16, kind="ExternalInput")
    xs = nc.dram_tensor("xs", [D, 512], bf16, kind="ExternalInput")
    lpT = nc.dram_tensor("lpT", [LAT, PAST], bf16, kind="ExternalInput")
    wq = nc.dram_tensor("wq", [D, LAT], bf16, kind="ExternalInput")
    wd = nc.dram_tensor("wd", [D, LAT], bf16, kind="ExternalInput")
    wk = nc.dram_tensor("wk", [LAT, LAT], bf16, kind="ExternalInput")
    wv = nc.dram_tensor("wv", [LAT, LAT], bf16, kind="ExternalInput")
    wo = nc.dram_tensor("wo", [LAT, D], bf16, kind="ExternalInput")
    g = nc.dram_tensor("g", [LAT], f32, kind="ExternalInput")
    b = nc.dram_tensor("bb", [LAT], f32, kind="ExternalInput")
    mask = nc.dram_tensor("mask", [128, 640], bf16, kind="ExternalInput")
    o = nc.dram_tensor("o", [T, D], f32, kind="ExternalOutput")

    with tile.TileContext(nc) as tc:
        with (
            tc.tile_pool(name="consts", bufs=1) as consts,
            tc.tile_pool(name="persist", bufs=1) as persist,
            tc.tile_pool(name="dramp", bufs=1, space="DRAM") as dramp,
        ):
            # mask[jj, c] = 1 iff c >= jj + 384: cols [384, 512) are the
            # causal staircase for one 128-key diagonal block; cols
            # [512, 640) are all-ones (bf16 ones for the diag denominator).
            # DMAs for these are issued inside phase A after the first
            # weight/x chunks, so the PE can start ~2us in.
            mask_sb = consts.tile([128, 640], bf16)
            ones_bf = mask_sb[:, 512:640]
            ones8 = consts.tile([128, 2, 128], f8)
            nc.vector.memset(ones8, 1.0)
            g_sb = consts.tile([128, NLC], f32)
            b_sb = consts.tile([128, NLC], f32)
            eps_sb = consts.tile([128, 1], f32)
            nc.vector.memset(eps_sb, LN_EPS)
            warm = consts.tile([128, 1], f32)
            nc.scalar.activation(warm, eps_sb, AF.Sqrt)
            nc.scalar.activation(warm, eps_sb, AF.Exp)
            wk_sb = consts.tile([128, NLC, LAT], bf16)
            wv_sb = consts.tile([128, NLC, LAT], bf16)
            lp_sb = consts.tile([128, NLC, PAST], bf16)

            def consts_dmas_early():
                nc.sync.dma_start(out=mask_sb, in_=mask[:, :])
                nc.sync.dma_start(
                    out=g_sb, in_=g[:].rearrange("(lc p) -> p lc", p=128)
                )
                nc.sync.dma_start(
                    out=b_sb, in_=b[:].rearrange("(lc p) -> p lc", p=128)
                )

            def consts_dmas_late(step):
                if step == 0:
                    nc.sync.dma_start(
                        out=wk_sb,
                        in_=wk[:, :].rearrange("(lc p) n -> p lc n", p=128),
                    )
                elif step == 1:
                    nc.sync.dma_start(
                        out=wv_sb,
                        in_=wv[:, :].rearrange("(lc p) n -> p lc n", p=128),
                    )
                else:
                    lc = step - 2
                    nc.sync.dma_start(
                        out=lp_sb[:, lc, :],
                        in_=lpT[lc * 128 : (lc + 1) * 128, :],
                    )

            qT_sb = persist.tile([128, HPC, T], bf16)  # q transposed, per head
            latn_sb = persist.tile([128, NLC, T], bf16)  # new latent, transposed
            kT_sb = persist.tile([128, HPC, S], bf16)  # [hd, h, s]
            vH_sb = persist.tile([128, NJB, HPC * 128], bf16)  # [s, jb, h*128+hd]
            ao_sb = persist.tile([128, HPC, T], bf16)  # attn out transposed

            # ---- Phase A: latent down-projection (this core's query-tile
            # slice only) + LayerNorm + AllGather, then q projection.
            # The d-block runs first so the AllGather's ~67us latency hides
            # behind the q projection and phase B-past.
            with (
                tc.tile_pool(name="wA", bufs=1) as wA,
                tc.tile_pool(name="wdt", bufs=3) as wdtp,
                tc.tile_pool(name="xa", bufs=5) as xa,
                tc.tile_pool(name="stats", bufs=1) as stats,
                tc.tile_pool(name="psA", bufs=1, space="PSUM") as psA,
            ):
                wq_sb = wA.tile([128, NDC, LAT], bf16)
                d_ps = [
                    psA.tile([128, 512], f32, tag=f"d{lc}", name=f"d_ps{lc}")
                    for lc in range(NLC)
                ]
                for dc in range(NDC):
                    if dc % 2 == 0:
                        c2 = slice(dc * 128, (dc + 2) * 128)
                        xst2 = xa.tile(
                            [128, 2, 512],
                            bf16,
                            tag=f"xs{(dc // 2) % 4}",
                            bufs=1,
                            name="xst2",
                        )
                        nc.sync.dma_start(
                            out=xst2,
                            in_=xs[c2, :].rearrange("(dc p) n -> p dc n", p=128),
                        )
                        wdt2 = wdtp.tile(
                            [128, 2, 512], bf16, tag="wdt", bufs=4, name="wdt2"
                        )
                        nc.scalar.dma_start(
                            out=wdt2,
                            in_=wd[c2, :].rearrange("(dc p) n -> p dc n", p=128),
                        )
                    if dc == 8:
                        nc.sync.dma_start(
                            out=wq_sb,
                            in_=wq[:, :].rearrange("(dc p) n -> p dc n", p=128),
                        )
                    if dc == 12:
                        consts_dmas_early()
                    for lc in range(NLC):
                        nc.tensor.matmul(
                            d_ps[lc],
                            lhsT=wdt2[:, dc % 2, lc * 128 : (lc + 1) * 128],
                            rhs=xst2[:, dc % 2, :],
                            start=(dc == 0),
                            stop=(dc == NDC - 1),
                        )
                # LayerNorm stats over the 512 latent dims (4 tiles x 128
                # partitions). Sums and squares read PSUM directly to shorten
                # the latency chain into the collective. Pack [sum | sumsq]
                # side by side; one gpsimd cross-partition all-reduce
                # produces both, replicated.
                # engines may read only ONE psum operand per instruction:
                # stage d_ps[1..3] into SBUF; d_ps[0] is read directly.
                ss2 = stats.tile([128, 1024], f32, tag="ss2", name="ss2")
                dn = [None] + [
                    stats.tile([128, 512], f32, tag=f"dn{i}", name=f"dn{i}")
                    for i in (1, 2, 3)
                ]
                for i in (1, 2, 3):
                    nc.scalar.copy(out=dn[i], in_=d_ps[i])
                sqs = [
                    stats.tile([128, 512], f32, tag=f"sq{i}", name=f"sq{i}")
                    for i in range(4)
                ]
                nc.scalar.square(sqs[0], d_ps[0])
                nc.scalar.square(sqs[1], dn[1])
                nc.vector.tensor_add(ss2[:, 0:512], d_ps[0], dn[1])
                nc.vector.tensor_add(ss2[:, 0:512], ss2[:, 0:512], dn[2])
                nc.vector.tensor_add(ss2[:, 0:512], ss2[:, 0:512], dn[3])
                nc.vector.tensor_mul(sqs[2], dn[2], dn[2])
                nc.vector.tensor_mul(sqs[3], dn[3], dn[3])
                nc.vector.tensor_add(ss2[:, 512:1024], sqs[0], sqs[1])
                nc.vector.tensor_add(ss2[:, 512:1024], ss2[:, 512:1024], sqs[2])
                nc.vector.tensor_add(ss2[:, 512:1024], ss2[:, 512:1024], sqs[3])
                pr2 = stats.tile([128, 1024], f32, tag="pr2", name="pr2")
                nc.gpsimd.partition_all_reduce(
                    pr2, ss2, channels=128, reduce_op=bass_isa.ReduceOp.add
                )
                mu = stats.tile([128, 512], f32, tag="mu", name="mu")
                nc.vector.tensor_scalar_mul(mu, pr2[:, 0:512], 1.0 / LAT)
                vtmp = stats.tile([128, 512], f32, tag="sq0", name="vtmp")
                nc.vector.tensor_mul(vtmp, mu, mu)
                sd = stats.tile([128, 512], f32, tag="sq1", name="sd")
                nc.vector.scalar_tensor_tensor(
                    out=sd,
                    in0=pr2[:, 512:1024],
                    scalar=1.0 / LAT,
                    in1=vtmp,
                    op0=OP.mult,
                    op1=OP.subtract,
                )
                nc.scalar.activation(sd, sd, AF.Sqrt, bias=eps_sb)
                rstd = stats.tile([128, 512], f32, tag="rstd", name="rstd")
                nc.vector.reciprocal_approx_fast(rstd, sd)
                lat_stage = stats.tile(
                    [128, NLC, 512], bf16, tag="lstg", name="lat_stage"
                )
                for lc in range(NLC):
                    eng = nc.vector if lc < 2 else nc.gpsimd
                    t1 = stats.tile(
                        [128, 512], f32, tag=f"sq{lc % 2}", name="t1"
                    )
                    eng.tensor_sub(t1, d_ps[lc] if lc == 0 else dn[lc], mu)
                    t2 = stats.tile(
                        [128, 512], f32, tag=f"sq{2 + lc % 2}", name="t2"
                    )
                    eng.tensor_mul(t2, t1, rstd)
                    eng.tensor_scalar(
                        lat_stage[:, lc, :],
                        t2,
                        g_sb[:, lc : lc + 1],
                        b_sb[:, lc : lc + 1],
                        OP.mult,
                        OP.add,
                    )
                # AllGather the 4 query-tile slices of the new latent among
                # the 4 cores of this batch (the tensor-parallel group)
                latn_part = dramp.tile([128, NLC, 512], bf16)
                latn_all = dramp.tile([4, 128, NLC, 512], bf16)
                nc.gpsimd.dma_start(latn_part[:, :, :], lat_stage)
                nc.gpsimd.collective_compute(
                    "AllGather",
                    mybir.AluOpType.bypass,
                    replica_groups=[[0, 1, 2, 3], [4, 5, 6, 7]],
                    ins=[latn_part.opt()],
                    outs=[latn_all.opt()],
                )
                nc.gpsimd.dma_start(
                    latn_sb[:, :, :].rearrange("p lc (j n) -> p lc j n", j=4),
                    latn_all[:, :, :, :].rearrange("j p lc n -> p lc j n"),
                )
                # q projection
                for tt in range(NTT):
                    tsl = slice(tt * 512, (tt + 1) * 512)
                    q_ps = [
                        psA.tile([128, 512], f32, tag=f"q{qc}", name=f"q_ps{qc}")
                        for qc in range(HPC)
                    ]
                    for dc in range(NDC):
                        xt = xa.tile([128, 512], bf16, tag="xt", name="xt")
                        nc.sync.dma_start(
                            out=xt, in_=xT[dc * 128 : (dc + 1) * 128, tsl]
                        )
                        if tt >= 1 and dc in (0, 8):
                            consts_dmas_late((tt - 1) * 2 + (dc // 8))
                        for qc in range(HPC):
                            nc.tensor.matmul(
                                q_ps[qc],
                                lhsT=wq_sb[:, dc, qc * 128 : (qc + 1) * 128],
                                rhs=xt,
                                start=(dc == 0),
                                stop=(dc == NDC - 1),
                            )
                    for qc in range(HPC):
                        if qc % 2 == 0:
                            nc.vector.tensor_copy(qT_sb[:, qc, tsl], q_ps[qc])
                        else:
                            nc.scalar.copy(out=qT_sb[:, qc, tsl], in_=q_ps[qc])

            # wo prefetch overlaps phases B+C; pool stays open through D
            with tc.tile_pool(name="wD", bufs=1) as wD:
                wo_sb = wD.tile([128, HPC, D], bf16)
                for dt_ in range(D // 512):
                    nc.sync.dma_start(
                        out=wo_sb[:, :, dt_ * 512 : (dt_ + 1) * 512],
                        in_=wo[:, dt_ * 512 : (dt_ + 1) * 512].rearrange(
                            "(hc p) n -> p hc n", p=128
                        ),
                    )

                # ---- Phase B: k/v up-projection for all 4 heads
                with (
                    tc.tile_pool(name="psB", bufs=1, space="PSUM") as psB,
                ):
                    for g_ in range(S // 512):
                        ssl = slice(g_ * 512, (g_ + 1) * 512)
                        if g_ < PAST // 512:

                            def latf(lc, _g=g_, _ssl=ssl):
                                return lp_sb[:, lc, _ssl]

                        else:

                            def latf(lc, _g=g_):
                                return latn_sb[
                                    :, lc, (_g - NTT) * 512 : (_g - NTT + 1) * 512
                                ]

                        for h in range(HPC):
                            k_ps = psB.tile(
                                [128, 512], f32, tag="kps", bufs=2, name="k_ps"
                            )
                            for lc in range(NLC):
                                nc.tensor.matmul(
                                    k_ps,
                                    lhsT=wk_sb[:, lc, h * 128 : (h + 1) * 128],
                                    rhs=latf(lc),
                                    start=(lc == 0),
                                    stop=(lc == NLC - 1),
                                )
                            if h % 2 == 0:
                                nc.scalar.copy(out=kT_sb[:, h, ssl], in_=k_ps)
                            else:
                                nc.vector.tensor_copy(kT_sb[:, h, ssl], k_ps)
                        for j4 in range(4):
                            # all 4 heads in one 512-wide moving dim
                            v_ps = psB.tile(
                                [128, 512], f32, tag="vps", bufs=2, name="v_ps"
                            )
                            for lc in range(NLC):
                                nc.tensor.matmul(
                                    v_ps,
                                    lhsT=latf(lc)[:, j4 * 128 : (j4 + 1) * 128],
                                    rhs=wv_sb[:, lc, :],
                                    start=(lc == 0),
                                    stop=(lc == NLC - 1),
                                )
                            if j4 % 2 == 0:
                                nc.vector.tensor_copy(
                                    vH_sb[:, g_ * 4 + j4, :], v_ps
                                )
                            else:
                                nc.scalar.copy(
                                    out=vH_sb[:, g_ * 4 + j4, :], in_=v_ps
                                )

                # ---- Phase C+D: attention per (query tile, head), with the
                # output projection for each query tile interleaved right
                # after its 4 heads finish. The o-proj PE work fills the gaps
                # while the scalar engine (exp) paces the attention stream.
                with (
                    tc.tile_pool(name="pp", bufs=1) as pp,
                    tc.tile_pool(name="ctmp", bufs=2) as ctmp,
                    tc.tile_pool(name="ost", bufs=4) as ost,
                    tc.tile_pool(name="psC", bufs=1, space="PSUM") as psC,
                ):
                    pidx = 0
                    ojobs = []

                    def emit_ojobs(n, act_ok=False):
                        for _ in range(min(n, len(ojobs))):
                            dt_, tc_ = ojobs.pop(0)
                            o_ps = psC.tile(
                                [128, 512], f32, tag="ops", bufs=2, name="o_ps"
                            )
                            for hc in range(HPC):
                                nc.tensor.matmul(
                                    o_ps,
                                    lhsT=ao_sb[:, hc, tc_ * 128 : (tc_ + 1) * 128],
                                    rhs=wo_sb[:, hc, dt_ * 512 : (dt_ + 1) * 512],
                                    start=(hc == 0),
                                    stop=(hc == HPC - 1),
                                )
                            o_sb = ost.tile([128, 512], f32, tag="osb", name="o_sb")
                            if act_ok and tc_ % 2 == 0:
                                nc.scalar.copy(out=o_sb, in_=o_ps)
                            else:
                                nc.vector.tensor_copy(o_sb, o_ps)
                            nc.sync.dma_start(
                                out=o[
                                    tc_ * 128 : (tc_ + 1) * 128,
                                    dt_ * 512 : (dt_ + 1) * 512,
                                ],
                                in_=o_sb,
                            )

                    for tt in range(NTT):
                        tsl = slice(tt * 512, (tt + 1) * 512)
                        for h in range(HPC):
                            hsl = slice(h * 128, (h + 1) * 128)
                            # full key blocks go in PAIRS (two score matmuls
                            # into one 2-bank psum tile, ONE exp over 1024
                            # cols). First pair opens the psum groups
                            # full-width; the 4 diagonal blocks (trimmed to
                            # their causally-visible columns) go next.
                            fulls = list(range(NPB)) + [
                                NPB + jn for jn in range(4 * tt)
                            ]
                            prs = [
                                (fulls[2 * i], fulls[2 * i + 1])
                                for i in range(len(fulls) // 2)
                            ]
                            diags = [(NPB + 4 * tt + i, i * 128) for i in range(4)]
                            ad = psC.tile([128, 1024], f32, tag="ad", bufs=1, name="ad")
                            attn_ps = ad[:, 0:512]
                            den_ps = ad[:, 512:1024]
                            pdiag = pp.tile([128, 512], bf16, tag="pd", name="pdiag")
                            state = {"p8q": None}

                            def do_pair(fi, ja, jbb, _h=h, _tsl=tsl, _hsl=hsl,
                                        _ad=ad, _npr=len(prs), _state=state):
                                global _PIDX
                                sp2 = psC.tile(
                                    [128, 1024], f32, tag="sps", bufs=2, name="sp2"
                                )
                                for i, jj in enumerate((ja, jbb)):
                                    nc.tensor.matmul(
                                        sp2[:, i * 512 : (i + 1) * 512],
                                        lhsT=kT_sb[:, _h, jj * 128 : (jj + 1) * 128],
                                        rhs=qT_sb[:, _h, _tsl],
                                        start=True,
                                        stop=True,
                                    )
                                p2 = pp.tile(
                                    [128, 2, 512], bf16, tag=f"p{_PIDX % 4}",
                                    name="p2",
                                )
                                _PIDX += 1
                                nc.scalar.activation(
                                    p2[:, :, :], sp2, AF.Exp, scale=SCALE
                                )
                                for i, jj in enumerate((ja, jbb)):
                                    nc.tensor.matmul(
                                        _ad[:, 0:512],
                                        lhsT=vH_sb[:, jj, _hsl],
                                        rhs=p2[:, i, :],
                                        start=(fi == 0 and i == 0),
                                        stop=(fi == _npr - 1 and i == 1),
                                    )
                                if fi % 2 == 0:
                                    _state["p8q"] = pp.tile(
                                        [128, 2, 512],
                                        f8,
                                        tag=f"p8q{(fi // 2) % 2}",
                                        name="p8q",
                                    )
                                # alternate Pool/DVE for the fp8 pair-sums
                                eng = nc.gpsimd if fi % 2 == 0 else nc.vector
                                eng.tensor_add(
                                    _state["p8q"][:, fi % 2, :], p2[:, 0, :],
                                    p2[:, 1, :],
                                )
                                if fi % 2 == 1:
                                    nc.tensor.matmul(
                                        _ad[:, 512:1024],
                                        lhsT=ones8,
                                        rhs=_state["p8q"],
                                        start=(fi == 1),
                                        stop=False,
                                        perf_mode=DR,
                                    )

                            do_pair(0, *prs[0])
                            for jb, rr in diags:
                                s_ps = psC.tile(
                                    [128, 1024], f32, tag="sps", bufs=2, name="s_ps"
                                )
                                nc.tensor.matmul(
                                    s_ps[:, rr:512],
                                    lhsT=kT_sb[:, h, jb * 128 : (jb + 1) * 128],
                                    rhs=qT_sb[:, h, tt * 512 + rr : (tt + 1) * 512],
                                    start=True,
                                    stop=True,
                                )
                                p_sb = pp.tile(
                                    [128, 512], bf16, tag=f"pdg{rr // 128 % 2}",
                                    name="p_sb",
                                )
                                nc.scalar.activation(
                                    p_sb[:, rr:], s_ps[:, rr:512], AF.Exp,
                                    scale=SCALE,
                                )
                                nc.vector.tensor_mul(
                                    p_sb[:, rr : rr + 128],
                                    p_sb[:, rr : rr + 128],
                                    mask_sb[:, 384:512],
                                )
                                nc.tensor.matmul(
                                    attn_ps[:, rr:],
                                    lhsT=vH_sb[:, jb, hsl],
                                    rhs=p_sb[:, rr:],
                                    start=False,
                                    stop=False,
                                )
                                if rr == 0:
                                    nc.vector.tensor_copy(pdiag, p_sb)
                                else:
                                    nc.vector.tensor_add(
                                        pdiag[:, rr:], pdiag[:, rr:], p_sb[:, rr:]
                                    )
                            emit_ojobs(2)
                            for fi in range(1, len(prs)):
                                do_pair(fi, *prs[fi])
                            # diagonal blocks' denominator contribution
                            nc.tensor.matmul(
                                den_ps,
                                lhsT=ones_bf,
                                rhs=pdiag,
                                start=False,
                                stop=True,
                            )
                            rec = ctmp.tile([128, 512], f32, tag="rec", name="rec")
                            nc.vector.reciprocal_approx_fast(rec, den_ps)
                            nc.vector.tensor_mul(ao_sb[:, h, tsl], attn_ps, rec)
                            emit_ojobs(3)
                        # queue this query tile's output-projection tiles;
                        # they are drained 5-at-a-time after later attention
                        # stages so their PE work absorbs exp (ACT) pacing.
                        ojobs.extend(
                            (dt_, tc_)
                            for dt_ in range(D // 512)
                            for tc_ in range(tt * 4, tt * 4 + 4)
                        )
                    emit_ojobs(len(ojobs), act_ok=True)

    nc.compile()
    return nc


def _get_nc():
    if "nc" not in _CACHE:
        _CACHE["nc"] = _build()
    return _CACHE["nc"]


def _make_mask():
    # B[jj, c] = 1.0 iff c >= jj + 384 for c < 512; cols [512:640) are all
    # ones. Sliced at [384:512] it is the causal staircase of one diagonal
    # 128-key block: query col c sees key jj iff c >= jj.
    import ml_dtypes

    jj = np.arange(128)[:, None]
    cc = np.arange(640)[None, :]
    return (cc >= jj + 384).astype(ml_dtypes.bfloat16)


def _in_maps(x, latent_prev, Wq, Wdown, Wk_up, Wv_up, ln_g, ln_b, Wo):
    import ml_dtypes

    bf = lambda a: np.ascontiguousarray(
        np.asarray(a, dtype=np.float32).astype(ml_dtypes.bfloat16)
    )
    f = lambda a: np.ascontiguousarray(np.asarray(a, dtype=np.float32))
    mask = _make_mask()
    maps = []
    for bi in range(2):
        xTb = bf(np.asarray(x)[bi].T)
        lpTb = bf(np.asarray(latent_prev)[bi].T)
        for hg in range(4):
            sl = slice(hg * 512, (hg + 1) * 512)
            maps.append(
                {
                    "xT": xTb,
                    "xs": np.ascontiguousarray(xTb[:, sl]),
                    "lpT": lpTb,
                    "wq": bf(np.asarray(Wq)[:, sl]),
                    "wd": bf(Wdown),
                    "wk": bf(np.asarray(Wk_up)[:, sl]),
                    "wv": bf(np.asarray(Wv_up)[:, sl]),
                    "wo": bf(np.asarray(Wo)[sl, :]),
                    "g": f(ln_g),
                    "bb": f(ln_b),
                    "mask": mask,
                }
            )
    return maps


def run(trace=False, **inputs):
    from concourse.bass_utils import run_bass_kernel_spmd

    nc = _get_nc()
    maps = _in_maps(**inputs)
    res = run_bass_kernel_spmd(nc, maps, core_ids=list(range(8)), trace=trace)
    outs = [res.results[c]["o"] for c in range(8)]
    out = np.stack(
        [
            outs[0] + outs[1] + outs[2] + outs[3],
            outs[4] + outs[5] + outs[6] + outs[7],
        ],
        axis=0,
    ).astype(np.float32)
    return out, res


def kernel(**inputs):
    out, _ = run(trace=False, **inputs)
    return out


# revision 30
# speedup vs baseline: 1.0351x; 1.0351x over previous
"""MultiHeadLatentAttention on 8 Trainium2 NeuronCores.

Sharding: 2 batches x 4 head-groups (4 heads each) = 8 cores.
Each core computes, for its batch b and heads [4*hg, 4*hg+4):
  q = x[b] @ Wq[:, cols]                  (computed transposed: qT [512, T])
  latent_new = LN(x[b] @ Wdown)           (computed transposed, replicated on
                                           the 4 cores of the same batch)
  kT = (latent @ Wk[:, cols]).T           v = latent @ Wv[:, cols]
  scores.T, softmax (no max-subtraction; |scores| <= ~3), PV accumulation
  o_partial = attn_out @ Wo[rows, :]      -> [T, D] partial sum
Host sums the 4 partials per batch and stacks the 2 batches.

Datapath is bf16 (same full PE rate as float32r, halves DMA + SBUF),
with fp32 PSUM accumulation and fp32 LayerNorm math; rel err ~4e-3.

Structure notes:
- k/v up-projection is hoisted out of the per-head loop; the v matmuls
  produce all 4 heads per instruction (moving dim 512) -- fp32r/bf16
  matmuls with moving dim 128 run at 1/4 rate, which made the baseline's
  per-head v-up cost 4x what it should.
- softmax denominator: full key blocks are pair-summed (Pool/DVE
  alternating) into fp8e4 pair tiles; one fp8 DoubleRow matmul per QUAD
  of blocks (0.5 cycles/row) replaces four full-rate ones-matmuls.
  Errors average out across thousands of positive terms.
- causally-dead key columns of the 4 diagonal key blocks per query tile
  are trimmed from scores/exp/PV (partial moving dim); their denominator
  contribution is accumulated separately in bf16 and added with one
  bf16 ones-matmul.
"""

import numpy as np

N_HEADS = 16
T = 2048
D = 2048
LAT = 512
PAST = 2048
S = PAST + T  # 4096, below the 8192 cache cap
HD = D // N_HEADS  # 128
HPC = 4  # heads per core
LN_EPS = 1e-5
SCALE = 1.0 / float(np.sqrt(HD))
NJB = S // 128  # 32 key blocks
NPB = PAST // 128  # 16 past key blocks
NTT = T // 512  # 4 query tiles
NDC = D // 128  # 16
NLC = LAT // 128  # 4

_CACHE = {}
_PIDX = 0


def _build():
    import concourse.bacc as bacc
    import concourse.mybir as mybir
    import concourse.tile as tile
    from concourse import bass_isa

    f32 = mybir.dt.float32
    bf16 = mybir.dt.bfloat16
    f8 = mybir.dt.float8e4
    AF = mybir.ActivationFunctionType
    OP = mybir.AluOpType
    DR = mybir.MatmulPerfMode.DoubleRow

    nc = bacc.Bacc("TRN2", target_bir_lowering=False, debug=False, num_devices=8)

    xT = nc.dram_tensor("xT", [D, T], bf16, kind="ExternalInput")
    xs = nc.dram_tensor("xs", [D, 512], bf16, kind="ExternalInput")
    lpT = nc.dram_tensor("lpT", [LAT, PAST], bf16, kind="ExternalInput")
    wq = nc.dram_tensor("wq", [D, LAT], bf16, kind="ExternalInput")
    wd = nc.dram_tensor("wd", [D, LAT], bf16, kind="ExternalInput")
    wk = nc.dram_tensor("wk", [LAT, LAT], bf16, kind="ExternalInput")
    wv = nc.dram_tensor("wv", [LAT, LAT], bf16, kind="ExternalInput")
    wo = nc.dram_tensor("wo", [LAT, D], bf16, kind="ExternalInput")
    g = nc.dram_tensor("g", [LAT], f32, kind="ExternalInput")
    b = nc.dram_tensor("bb", [LAT], f32, kind="ExternalInput")
    mask = nc.dram_tensor("mask", [128, 640], bf16, kind="ExternalInput")
    o = nc.dram_tensor("o", [T, D], f32, kind="ExternalOutput")

    with tile.TileContext(nc) as tc:
        with (
            tc.tile_pool(name="consts", bufs=1) as consts,
            tc.tile_pool(name="persist", bufs=1) as persist,
            tc.tile_pool(name="dramp", bufs=1, space="DRAM") as dramp,
        ):
            # mask[jj, c] = 1 iff c >= jj + 384: cols [384, 512) are the
            # causal staircase for one 128-key diagonal block; cols
            # [512, 640) are all-ones (bf16 ones for the diag denominator).
            # DMAs for these are issued inside phase A after the first
            # weight/x chunks, so the PE can start ~2us in.
            mask_sb = consts.tile([128, 640], bf16)
            ones_bf = mask_sb[:, 512:640]
            ones8 = consts.tile([128, 2, 128], f8)
            nc.vector.memset(ones8, 1.0)
            g_sb = consts.tile([128, NLC], f32)
            b_sb = consts.tile([128, NLC], f32)
            eps_sb = consts.tile([128, 1], f32)
            nc.vector.memset(eps_sb, LN_EPS)
            warm = consts.tile([128, 1], f32)
            nc.scalar.activation(warm, eps_sb, AF.Sqrt)
            nc.scalar.activation(warm, eps_sb, AF.Exp)
            wk_sb = consts.tile([128, NLC, LAT], bf16)
            wv_sb = consts.tile([128, NLC, LAT], bf16)
            lp_sb = consts.tile([128, NLC, PAST], bf16)

            def consts_dmas_early():
                nc.sync.dma_start(out=mask_sb, in_=mask[:, :])
                nc.sync.dma_start(
                    out=g_sb, in_=g[:].rearrange("(lc p) -> p lc", p=128)
                )
                nc.sync.dma_start(
                    out=b_sb, in_=b[:].rearrange("(lc p) -> p lc", p=128)
                )

            def consts_dmas_late(step):
                if step == 0:
                    nc.sync.dma_start(
                        out=wk_sb,
                        in_=wk[:, :].rearrange("(lc p) n -> p lc n", p=128),
                    )
                elif step == 1:
                    nc.sync.dma_start(
                        out=wv_sb,
                        in_=wv[:, :].rearrange("(lc p) n -> p lc n", p=128),
                    )
                else:
                    lc = step - 2
                    nc.sync.dma_start(
                        out=lp_sb[:, lc, :],
                        in_=lpT[lc * 128 : (lc + 1) * 128, :],
                    )

            qT_sb = persist.tile([128, HPC, T], bf16)  # q transposed, per head
            latn_sb = persist.tile([128, NLC, T], bf16)  # new latent, transposed
            kT_sb = persist.tile([128, HPC, S], bf16)  # [hd, h, s]
            vH_sb = persist.tile([128, NJB, HPC * 128], bf16)  # [s, jb, h*128+hd]
            ao_sb = persist.tile([128, HPC, T], bf16)  # attn out transposed

            # ---- Phase A: latent down-projection (this core's query-tile
            # slice only) + LayerNorm + AllGather, then q projection.
            # The d-block runs first so the AllGather's ~67us latency hides
            # behind the q projection and phase B-past.
            with (
                tc.tile_pool(name="wA", bufs=1) as wA,
                tc.tile_pool(name="wdt", bufs=3) as wdtp,
                tc.tile_pool(name="xa", bufs=5) as xa,
                tc.tile_pool(name="stats", bufs=1) as stats,
                tc.tile_pool(name="psA", bufs=1, space="PSUM") as psA,
            ):
                wq_sb = wA.tile([128, NDC, LAT], bf16)
                d_ps = [
                    psA.tile([128, 512], f32, tag=f"d{lc}", name=f"d_ps{lc}")
                    for lc in range(NLC)
                ]
                for dc in range(NDC):
                    if dc % 2 == 0:
                        c2 = slice(dc * 128, (dc + 2) * 128)
                        xst2 = xa.tile(
                            [128, 2, 512],
                            bf16,
                            tag=f"xs{(dc // 2) % 4}",
                            bufs=1,
                            name="xst2",
                        )
                        nc.sync.dma_start(
                            out=xst2,
                            in_=xs[c2, :].rearrange("(dc p) n -> p dc n", p=128),
                        )
                        wdt2 = wdtp.tile(
                            [128, 2, 512], bf16, tag="wdt", bufs=4, name="wdt2"
                        )
                        nc.scalar.dma_start(
                            out=wdt2,
                            in_=wd[c2, :].rearrange("(dc p) n -> p dc n", p=128),
                        )
                    if dc == 8:
                        nc.sync.dma_start(
                            out=wq_sb,
                            in_=wq[:, :].rearrange("(dc p) n -> p dc n", p=128),
                        )
                    if dc == 12:
                        consts_dmas_early()
                    for lc in range(NLC):
                        nc.tensor.matmul(
                            d_ps[lc],
                            lhsT=wdt2[:, dc % 2, lc * 128 : (lc + 1) * 128],
                            rhs=xst2[:, dc % 2, :],
                            start=(dc == 0),
                            stop=(dc == NDC - 1),
                        )
                # LayerNorm stats over the 512 latent dims (4 tiles x 128
                # partitions). Sums and squares read PSUM directly to shorten
                # the latency chain into the collective. Pack [sum | sumsq]
                # side by side; one gpsimd cross-partition all-reduce
                # produces both, replicated.
                # engines may read only ONE psum operand per instruction:
                # stage d_ps[1..3] into SBUF; d_ps[0] is read directly.
                ss2 = stats.tile([128, 1024], f32, tag="ss2", name="ss2")
                dn = [None] + [
                    stats.tile([128, 512], f32, tag=f"dn{i}", name=f"dn{i}")
                    for i in (1, 2, 3)
                ]
                for i in (1, 2, 3):
                    nc.scalar.copy(out=dn[i], in_=d_ps[i])
                sqs = [
                    stats.tile([128, 512], f32, tag=f"sq{i}", name=f"sq{i}")
                    for i in range(4)
                ]
                nc.scalar.square(sqs[0], d_ps[0])
                nc.scalar.square(sqs[1], dn[1])
                nc.vector.tensor_add(ss2[:, 0:512], d_ps[0], dn[1])
                nc.vector.tensor_add(ss2[:, 0:512], ss2[:, 0:512], dn[2])
                nc.vector.tensor_add(ss2[:, 0:512], ss2[:, 0:512], dn[3])
                nc.vector.tensor_mul(sqs[2], dn[2], dn[2])
                nc.vector.tensor_mul(sqs[3], dn[3], dn[3])
                nc.vector.tensor_add(ss2[:, 512:1024], sqs[0], sqs[1])
                nc.vector.tensor_add(ss2[:, 512:1024], ss2[:, 512:1024], sqs[2])
                nc.vector.tensor_add(ss2[:, 512:1024], ss2[:, 512:1024], sqs[3])
                pr2 = stats.tile([128, 1024], f32, tag="pr2", name="pr2")
                nc.gpsimd.partition_all_reduce(
                    pr2, ss2, channels=128, reduce_op=bass_isa.ReduceOp.add
                )
                mu = stats.tile([128, 512], f32, tag="mu", name="mu")
                nc.vector.tensor_scalar_mul(mu, pr2[:, 0:512], 1.0 / LAT)
                vtmp = stats.tile([128, 512], f32, tag="sq0", name="vtmp")
                nc.vector.tensor_mul(vtmp, mu, mu)
                sd = stats.tile([128, 512], f32, tag="sq1", name="sd")
                nc.vector.scalar_tensor_tensor(
                    out=sd,
                    in0=pr2[:, 512:1024],
                    scalar=1.0 / LAT,
                    in1=vtmp,
                    op0=OP.mult,
                    op1=OP.subtract,
                )
                nc.scalar.activation(sd, sd, AF.Sqrt, bias=eps_sb)
                rstd = stats.tile([128, 512], f32, tag="rstd", name="rstd")
                nc.vector.reciprocal_approx_fast(rstd, sd)
                lat_stage = stats.tile(
                    [128, NLC, 512], bf16, tag="lstg", name="lat_stage"
                )
                for lc in range(NLC):
                    eng = nc.vector if lc < 2 else nc.gpsimd
                    t1 = stats.tile(
                        [128, 512], f32, tag=f"sq{lc % 2}", name="t1"
                    )
                    eng.tensor_sub(t1, d_ps[lc] if lc == 0 else dn[lc], mu)
                    t2 = stats.tile(
                        [128, 512], f32, tag=f"sq{2 + lc % 2}", name="t2"
                    )
                    eng.tensor_mul(t2, t1, rstd)
                    eng.tensor_scalar(
                        lat_stage[:, lc, :],
                        t2,
                        g_sb[:, lc : lc + 1],
                        b_sb[:, lc : lc + 1],
                        OP.mult,
                        OP.add,
                    )
                # AllGather the 4 query-tile slices of the new latent among
                # the 4 cores of this batch (the tensor-parallel group)
                latn_part = dramp.tile([128, NLC, 512], bf16)
                latn_all = dramp.tile([4, 128, NLC, 512], bf16)
                nc.gpsimd.dma_start(latn_part[:, :, :], lat_stage)
                nc.gpsimd.collective_compute(
                    "AllGather",
                    mybir.AluOpType.bypass,
                    replica_groups=[[0, 1, 2, 3], [4, 5, 6, 7]],
                    ins=[latn_part.opt()],
                    outs=[latn_all.opt()],
                )
                nc.gpsimd.dma_start(
                    latn_sb[:, :, :].rearrange("p lc (j n) -> p lc j n", j=4),
                    latn_all[:, :, :, :].rearrange("j p lc n -> p lc j n"),
                )
                # q projection
                for tt in range(NTT):
                    tsl = slice(tt * 512, (tt + 1) * 512)
                    q_ps = [
                        psA.tile([128, 512], f32, tag=f"q{qc}", name=f"q_ps{qc}")
                        for qc in range(HPC)
                    ]
                    for dc in range(NDC):
                        xt = xa.tile([128, 512], bf16, tag="xt", name="xt")
                        nc.sync.dma_start(
                            out=xt, in_=xT[dc * 128 : (dc + 1) * 128, tsl]
                        )
                        if tt >= 1 and dc in (0, 8):
                            consts_dmas_late((tt - 1) * 2 + (dc // 8))
                        for qc in range(HPC):
                            nc.tensor.matmul(
                                q_ps[qc],
                                lhsT=wq_sb[:, dc, qc * 128 : (qc + 1) * 128],
                                rhs=xt,
                                start=(dc == 0),
                                stop=(dc == NDC - 1),
                            )
                    for qc in range(HPC):
                        if qc % 2 == 0:
                            nc.vector.tensor_copy(qT_sb[:, qc, tsl], q_ps[qc])
                        else:
                            nc.scalar.copy(out=qT_sb[:, qc, tsl], in_=q_ps[qc])

            # wo prefetch overlaps phases B+C; pool stays open through D
            with tc.tile_pool(name="wD", bufs=1) as wD:
                wo_sb = wD.tile([128, HPC, D], bf16)
                for dt_ in range(D // 512):
                    nc.sync.dma_start(
                        out=wo_sb[:, :, dt_ * 512 : (dt_ + 1) * 512],
                        in_=wo[:, dt_ * 512 : (dt_ + 1) * 512].rearrange(
                            "(hc p) n -> p hc n", p=128
                        ),
                    )

                # ---- Phase B: k/v up-projection for all 4 heads
                with (
                    tc.tile_pool(name="psB", bufs=1, space="PSUM") as psB,
                ):
                    for g_ in range(PAST // 512):
                        ssl = slice(g_ * 512, (g_ + 1) * 512)

                        def latf(lc, _g=g_, _ssl=ssl):
                            return lp_sb[:, lc, _ssl]

                        for h in range(HPC):
                            k_ps = psB.tile(
                                [128, 512], f32, tag="kps", bufs=2, name="k_ps"
                            )
                            for lc in range(NLC):
                                nc.tensor.matmul(
                                    k_ps,
                                    lhsT=wk_sb[:, lc, h * 128 : (h + 1) * 128],
                                    rhs=latf(lc),
                                    start=(lc == 0),
                                    stop=(lc == NLC - 1),
                                )
                            if h % 2 == 0:
                                nc.scalar.copy(out=kT_sb[:, h, ssl], in_=k_ps)
                            else:
                                nc.vector.tensor_copy(kT_sb[:, h, ssl], k_ps)
                        for j4 in range(4):
                            # all 4 heads in one 512-wide moving dim
                            v_ps = psB.tile(
                                [128, 512], f32, tag="vps", bufs=2, name="v_ps"
                            )
                            for lc in range(NLC):
                                nc.tensor.matmul(
                                    v_ps,
                                    lhsT=latf(lc)[:, j4 * 128 : (j4 + 1) * 128],
                                    rhs=wv_sb[:, lc, :],
                                    start=(lc == 0),
                                    stop=(lc == NLC - 1),
                                )
                            if j4 % 2 == 0:
                                nc.vector.tensor_copy(
                                    vH_sb[:, g_ * 4 + j4, :], v_ps
                                )
                            else:
                                nc.scalar.copy(
                                    out=vH_sb[:, g_ * 4 + j4, :], in_=v_ps
                                )

                # ---- Phase C+D: attention per (query tile, head), with the
                # output projection for each query tile interleaved right
                # after its 4 heads finish. The o-proj PE work fills the gaps
                # while the scalar engine (exp) paces the attention stream.
                with (
                    tc.tile_pool(name="pp", bufs=1) as pp,
                    tc.tile_pool(name="ctmp", bufs=2) as ctmp,
                    tc.tile_pool(name="ost", bufs=4) as ost,
                    tc.tile_pool(name="psC", bufs=1, space="PSUM") as psC,
                ):
                    pidx = 0
                    ojobs = []

                    def emit_bnew(g_):
                        ssl = slice(g_ * 512, (g_ + 1) * 512)
                        nsl = slice((g_ - NTT) * 512, (g_ - NTT + 1) * 512)
                        for h in range(HPC):
                            k_ps = psC.tile(
                                [128, 512], f32, tag="ops", bufs=2, name="k_ps"
                            )
                            for lc in range(NLC):
                                nc.tensor.matmul(
                                    k_ps,
                                    lhsT=wk_sb[:, lc, h * 128 : (h + 1) * 128],
                                    rhs=latn_sb[:, lc, nsl],
                                    start=(lc == 0),
                                    stop=(lc == NLC - 1),
                                )
                            if h % 2 == 0:
                                nc.scalar.copy(out=kT_sb[:, h, ssl], in_=k_ps)
                            else:
                                nc.vector.tensor_copy(kT_sb[:, h, ssl], k_ps)
                        for j4 in range(4):
                            v_ps = psC.tile(
                                [128, 512], f32, tag="ops", bufs=2, name="v_ps"
                            )
                            for lc in range(NLC):
                                nc.tensor.matmul(
                                    v_ps,
                                    lhsT=latn_sb[:, lc, nsl][
                                        :, j4 * 128 : (j4 + 1) * 128
                                    ],
                                    rhs=wv_sb[:, lc, :],
                                    start=(lc == 0),
                                    stop=(lc == NLC - 1),
                                )
                            if j4 % 2 == 0:
                                nc.vector.tensor_copy(vH_sb[:, g_ * 4 + j4, :], v_ps)
                            else:
                                nc.scalar.copy(out=vH_sb[:, g_ * 4 + j4, :], in_=v_ps)

                    def emit_ojobs(n, act_ok=False):
                        for _ in range(min(n, len(ojobs))):
                            dt_, tc_ = ojobs.pop(0)
                            o_ps = psC.tile(
                                [128, 512], f32, tag="ops", bufs=2, name="o_ps"
                            )
                            for hc in range(HPC):
                                nc.tensor.matmul(
                                    o_ps,
                                    lhsT=ao_sb[:, hc, tc_ * 128 : (tc_ + 1) * 128],
                                    rhs=wo_sb[:, hc, dt_ * 512 : (dt_ + 1) * 512],
                                    start=(hc == 0),
                                    stop=(hc == HPC - 1),
                                )
                            o_sb = ost.tile([128, 512], f32, tag="osb", name="o_sb")
                            if act_ok and tc_ % 2 == 0:
                                nc.scalar.copy(out=o_sb, in_=o_ps)
                            else:
                                nc.vector.tensor_copy(o_sb, o_ps)
                            nc.sync.dma_start(
                                out=o[
                                    tc_ * 128 : (tc_ + 1) * 128,
                                    dt_ * 512 : (dt_ + 1) * 512,
                                ],
                                in_=o_sb,
                            )

                    for tt in range(NTT):
                        tsl = slice(tt * 512, (tt + 1) * 512)
                        for h in range(HPC):
                            hsl = slice(h * 128, (h + 1) * 128)
                            # the very first stage runs all its past pairs
                            # BEFORE the deferred g4 k/v group, hiding the
                            # latent AllGather latency; its diagonal blocks
                            # (which need g4) then close the psum groups.
                            diags_last = tt == 0 and h == 0
                            # full key blocks go in PAIRS (two score matmuls
                            # into one 2-bank psum tile, ONE exp over 1024
                            # cols). First pair opens the psum groups
                            # full-width; the 4 diagonal blocks (trimmed to
                            # their causally-visible columns) go next.
                            fulls = list(range(NPB)) + [
                                NPB + jn for jn in range(4 * tt)
                            ]
                            prs = [
                                (fulls[2 * i], fulls[2 * i + 1])
                                for i in range(len(fulls) // 2)
                            ]
                            diags = [(NPB + 4 * tt + i, i * 128) for i in range(4)]
                            ad = psC.tile([128, 1024], f32, tag="ad", bufs=1, name="ad")
                            attn_ps = ad[:, 0:512]
                            den_ps = ad[:, 512:1024]
                            pdiag = pp.tile([128, 512], bf16, tag="pd", name="pdiag")
                            state = {"p8q": None, "dr": None}

                            def flush_dr(_ad=ad, _state=None):
                                st = _state if _state is not None else state
                                if st["dr"] is not None:
                                    q8, first = st["dr"]
                                    nc.tensor.matmul(
                                        _ad[:, 512:1024],
                                        lhsT=ones8,
                                        rhs=q8,
                                        start=first,
                                        stop=False,
                                        perf_mode=DR,
                                    )
                                    st["dr"] = None

                            def do_pair(fi, ja, jbb, _h=h, _tsl=tsl, _hsl=hsl,
                                        _ad=ad, _npr=len(prs), _state=state,
                                        _pv_stop=(not diags_last)):
                                global _PIDX
                                sp2 = psC.tile(
                                    [128, 1024], f32, tag="sps", bufs=2, name="sp2"
                                )
                                for i, jj in enumerate((ja, jbb)):
                                    nc.tensor.matmul(
                                        sp2[:, i * 512 : (i + 1) * 512],
                                        lhsT=kT_sb[:, _h, jj * 128 : (jj + 1) * 128],
                                        rhs=qT_sb[:, _h, _tsl],
                                        start=True,
                                        stop=True,
                                    )
                                flush_dr(_state=_state)
                                p2 = pp.tile(
                                    [128, 2, 512], bf16, tag=f"p{_PIDX % 4}",
                                    name="p2",
                                )
                                _PIDX += 1
                                nc.scalar.activation(
                                    p2[:, :, :], sp2, AF.Exp, scale=SCALE
                                )
                                for i, jj in enumerate((ja, jbb)):
                                    nc.tensor.matmul(
                                        _ad[:, 0:512],
                                        lhsT=vH_sb[:, jj, _hsl],
                                        rhs=p2[:, i, :],
                                        start=(fi == 0 and i == 0),
                                        stop=(
                                            _pv_stop
                                            and fi == _npr - 1
                                            and i == 1
                                        ),
                                    )
                                if fi % 2 == 0:
                                    _state["p8q"] = pp.tile(
                                        [128, 2, 512],
                                        f8,
                                        tag=f"p8q{(fi // 2) % 2}",
                                        name="p8q",
                                    )
                                # alternate Pool/DVE for the fp8 pair-sums
                                eng = nc.gpsimd if fi % 2 == 0 else nc.vector
                                eng.tensor_add(
                                    _state["p8q"][:, fi % 2, :], p2[:, 0, :],
                                    p2[:, 1, :],
                                )
                                if fi % 2 == 1:
                                    _state["dr"] = (_state["p8q"], fi == 1)

                            do_pair(0, *prs[0])
                            if diags_last:
                                for fi in range(1, len(prs)):
                                    do_pair(fi, *prs[fi])
                                emit_bnew(4)
                            ndiag = len(diags)
                            for di, (jb, rr) in enumerate(diags):
                                s_ps = psC.tile(
                                    [128, 1024], f32, tag="sps", bufs=2, name="s_ps"
                                )
                                nc.tensor.matmul(
                                    s_ps[:, rr:512],
                                    lhsT=kT_sb[:, h, jb * 128 : (jb + 1) * 128],
                                    rhs=qT_sb[:, h, tt * 512 + rr : (tt + 1) * 512],
                                    start=True,
                                    stop=True,
                                )
                                p_sb = pp.tile(
                                    [128, 512], bf16, tag=f"pdg{rr // 128 % 2}",
                                    name="p_sb",
                                )
                                nc.scalar.activation(
                                    p_sb[:, rr:], s_ps[:, rr:512], AF.Exp,
                                    scale=SCALE,
                                )
                                nc.vector.tensor_mul(
                                    p_sb[:, rr : rr + 128],
                                    p_sb[:, rr : rr + 128],
                                    mask_sb[:, 384:512],
                                )
                                nc.tensor.matmul(
                                    attn_ps[:, rr:],
                                    lhsT=vH_sb[:, jb, hsl],
                                    rhs=p_sb[:, rr:],
                                    start=False,
                                    stop=(diags_last and di == ndiag - 1),
                                )
                                if rr == 0:
                                    nc.vector.tensor_copy(pdiag, p_sb)
                                else:
                                    nc.vector.tensor_add(
                                        pdiag[:, rr:], pdiag[:, rr:], p_sb[:, rr:]
                                    )
                            emit_ojobs(2)
                            if not diags_last:
                                for fi in range(1, len(prs)):
                                    do_pair(fi, *prs[fi])
                            flush_dr()
                            # diagonal blocks' denominator contribution
                            nc.tensor.matmul(
                                den_ps,
                                lhsT=ones_bf,
                                rhs=pdiag,
                                start=False,
                                stop=True,
                            )
                            rec = ctmp.tile([128, 512], f32, tag="rec", name="rec")
                            nc.vector.reciprocal_approx_fast(rec, den_ps)
                            nc.vector.tensor_mul(ao_sb[:, h, tsl], attn_ps, rec)
                            emit_ojobs(3)
                            if tt == 0 and h < 3:
                                emit_bnew(5 + h)
                        # queue this query tile's output-projection tiles;
                        # they are drained 5-at-a-time after later attention
                        # stages so their PE work absorbs exp (ACT) pacing.
                        ojobs.extend(
                            (dt_, tc_)
                            for dt_ in range(D // 512)
                            for tc_ in range(tt * 4, tt * 4 + 4)
                        )
                    emit_ojobs(len(ojobs), act_ok=True)

    nc.compile()
    return nc


def _get_nc():
    if "nc" not in _CACHE:
        _CACHE["nc"] = _build()
    return _CACHE["nc"]


def _make_mask():
    # B[jj, c] = 1.0 iff c >= jj + 384 for c < 512; cols [512:640) are all
    # ones. Sliced at [384:512] it is the causal staircase of one diagonal
    # 128-key block: query col c sees key jj iff c >= jj.
    import ml_dtypes

    jj = np.arange(128)[:, None]
    cc = np.arange(640)[None, :]
    return (cc >= jj + 384).astype(ml_dtypes.bfloat16)


def _in_maps(x, latent_prev, Wq, Wdown, Wk_up, Wv_up, ln_g, ln_b, Wo):
    import ml_dtypes

    bf = lambda a: np.ascontiguousarray(
        np.asarray(a, dtype=np.float32).astype(ml_dtypes.bfloat16)
    )
    f = lambda a: np.ascontiguousarray(np.asarray(a, dtype=np.float32))
    mask = _make_mask()
    maps = []
    for bi in range(2):
        xTb = bf(np.asarray(x)[bi].T)
        lpTb = bf(np.asarray(latent_prev)[bi].T)
        for hg in range(4):
            sl = slice(hg * 512, (hg + 1) * 512)
            maps.append(
                {
                    "xT": xTb,
                    "xs": np.ascontiguousarray(xTb[:, sl]),
                    "lpT": lpTb,
                    "wq": bf(np.asarray(Wq)[:, sl]),
                    "wd": bf(Wdown),
                    "wk": bf(np.asarray(Wk_up)[:, sl]),
                    "wv": bf(np.asarray(Wv_up)[:, sl]),
                    "wo": bf(np.asarray(Wo)[sl, :]),
                    "g": f(ln_g),
                    "bb": f(ln_b),
                    "mask": mask,
                }
            )
    return maps


def run(trace=False, **inputs):
    from concourse.bass_utils import run_bass_kernel_spmd

    nc = _get_nc()
    maps = _in_maps(**inputs)
    res = run_bass_kernel_spmd(nc, maps, core_ids=list(range(8)), trace=trace)
    outs = [res.results[c]["o"] for c in range(8)]
    out = np.stack(
        [
            outs[0] + outs[1] + outs[2] + outs[3],
            outs[4] + outs[5] + outs[6] + outs[7],
        ],
        axis=0,
    ).astype(np.float32)
    return out, res


def kernel(**inputs):
    out, _ = run(trace=False, **inputs)
    return out


# revision 31
# speedup vs baseline: 1.0533x; 1.0176x over previous
"""MultiHeadLatentAttention on 8 Trainium2 NeuronCores.

Sharding: 2 batches x 4 head-groups (4 heads each) = 8 cores.
Each core computes, for its batch b and heads [4*hg, 4*hg+4):
  q = x[b] @ Wq[:, cols]                  (computed transposed: qT [512, T])
  latent_new = LN(x[b] @ Wdown)           (computed transposed, replicated on
                                           the 4 cores of the same batch)
  kT = (latent @ Wk[:, cols]).T           v = latent @ Wv[:, cols]
  scores.T, softmax (no max-subtraction; |scores| <= ~3), PV accumulation
  o_partial = attn_out @ Wo[rows, :]      -> [T, D] partial sum
Host sums the 4 partials per batch and stacks the 2 batches.

Datapath is bf16 (same full PE rate as float32r, halves DMA + SBUF),
with fp32 PSUM accumulation and fp32 LayerNorm math; rel err ~4e-3.

Structure notes:
- k/v up-projection is hoisted out of the per-head loop; the v matmuls
  produce all 4 heads per instruction (moving dim 512) -- fp32r/bf16
  matmuls with moving dim 128 run at 1/4 rate, which made the baseline's
  per-head v-up cost 4x what it should.
- softmax denominator: full key blocks are pair-summed (Pool/DVE
  alternating) into fp8e4 pair tiles; one fp8 DoubleRow matmul per QUAD
  of blocks (0.5 cycles/row) replaces four full-rate ones-matmuls.
  Errors average out across thousands of positive terms.
- causally-dead key columns of the 4 diagonal key blocks per query tile
  are trimmed from scores/exp/PV (partial moving dim); their denominator
  contribution is accumulated separately in bf16 and added with one
  bf16 ones-matmul.
"""

import numpy as np

N_HEADS = 16
T = 2048
D = 2048
LAT = 512
PAST = 2048
S = PAST + T  # 4096, below the 8192 cache cap
HD = D // N_HEADS  # 128
HPC = 4  # heads per core
LN_EPS = 1e-5
SCALE = 1.0 / float(np.sqrt(HD))
NJB = S // 128  # 32 key blocks
NPB = PAST // 128  # 16 past key blocks
NTT = T // 512  # 4 query tiles
NDC = D // 128  # 16
NLC = LAT // 128  # 4

_CACHE = {}
_PIDX = 0


def _build():
    import concourse.bacc as bacc
    import concourse.mybir as mybir
    import concourse.tile as tile
    from concourse import bass_isa

    f32 = mybir.dt.float32
    bf16 = mybir.dt.bfloat16
    f8 = mybir.dt.float8e4
    AF = mybir.ActivationFunctionType
    OP = mybir.AluOpType
    DR = mybir.MatmulPerfMode.DoubleRow

    nc = bacc.Bacc("TRN2", target_bir_lowering=False, debug=False, num_devices=8)

    xT = nc.dram_tensor("xT", [D, T], bf16, kind="ExternalInput")
    xs = nc.dram_tensor("xs", [D, 512], bf16, kind="ExternalInput")
    lpT = nc.dram_tensor("lpT", [LAT, PAST], bf16, kind="ExternalInput")
    wq = nc.dram_tensor("wq", [D, LAT], bf16, kind="ExternalInput")
    wd = nc.dram_tensor("wd", [D, LAT], bf16, kind="ExternalInput")
    wk = nc.dram_tensor("wk", [LAT, LAT], bf16, kind="ExternalInput")
    wv = nc.dram_tensor("wv", [LAT, LAT], bf16, kind="ExternalInput")
    wo = nc.dram_tensor("wo", [LAT, D], bf16, kind="ExternalInput")
    g = nc.dram_tensor("g", [LAT], f32, kind="ExternalInput")
    b = nc.dram_tensor("bb", [LAT], f32, kind="ExternalInput")
    mask = nc.dram_tensor("mask", [128, 640], bf16, kind="ExternalInput")
    o = nc.dram_tensor("o", [T, D], f32, kind="ExternalOutput")

    with tile.TileContext(nc) as tc:
        with (
            tc.tile_pool(name="consts", bufs=1) as consts,
            tc.tile_pool(name="persist", bufs=1) as persist,
            tc.tile_pool(name="dramp", bufs=1, space="DRAM") as dramp,
        ):
            # mask[jj, c] = 1 iff c >= jj + 384: cols [384, 512) are the
            # causal staircase for one 128-key diagonal block; cols
            # [512, 640) are all-ones (bf16 ones for the diag denominator).
            # DMAs for these are issued inside phase A after the first
            # weight/x chunks, so the PE can start ~2us in.
            mask_sb = consts.tile([128, 640], bf16)
            ones_bf = mask_sb[:, 512:640]
            ones8 = consts.tile([128, 2, 128], f8)
            nc.vector.memset(ones8, 1.0)
            g_sb = consts.tile([128, NLC], f32)
            b_sb = consts.tile([128, NLC], f32)
            eps_sb = consts.tile([128, 1], f32)
            nc.vector.memset(eps_sb, LN_EPS)
            warm = consts.tile([128, 1], f32)
            nc.scalar.activation(warm, eps_sb, AF.Sqrt)
            nc.scalar.activation(warm, eps_sb, AF.Exp)
            wk_sb = consts.tile([128, NLC, LAT], bf16)
            wv_sb = consts.tile([128, NLC, LAT], bf16)
            lp_sb = consts.tile([128, NLC, PAST], bf16)

            def consts_dmas_early():
                nc.sync.dma_start(out=mask_sb, in_=mask[:, :])
                nc.sync.dma_start(
                    out=g_sb, in_=g[:].rearrange("(lc p) -> p lc", p=128)
                )
                nc.sync.dma_start(
                    out=b_sb, in_=b[:].rearrange("(lc p) -> p lc", p=128)
                )

            def consts_dmas_late(step):
                if step == 0:
                    nc.sync.dma_start(
                        out=wk_sb,
                        in_=wk[:, :].rearrange("(lc p) n -> p lc n", p=128),
                    )
                elif step == 1:
                    nc.sync.dma_start(
                        out=wv_sb,
                        in_=wv[:, :].rearrange("(lc p) n -> p lc n", p=128),
                    )
                else:
                    lc = step - 2
                    nc.sync.dma_start(
                        out=lp_sb[:, lc, :],
                        in_=lpT[lc * 128 : (lc + 1) * 128, :],
                    )

            qT_sb = persist.tile([128, HPC, T], bf16)  # q transposed, per head
            latn_sb = persist.tile([128, NLC, T], bf16)  # new latent, transposed
            kT_sb = persist.tile([128, HPC, S], bf16)  # [hd, h, s]
            vH_sb = persist.tile([128, NJB, HPC * 128], bf16)  # [s, jb, h*128+hd]
            ao_sb = persist.tile([128, HPC, T], bf16)  # attn out transposed

            # ---- Phase A: latent down-projection (this core's query-tile
            # slice only) + LayerNorm + AllGather, then q projection.
            # The d-block runs first so the AllGather's ~67us latency hides
            # behind the q projection and phase B-past.
            with (
                tc.tile_pool(name="wA", bufs=1) as wA,
                tc.tile_pool(name="wdt", bufs=3) as wdtp,
                tc.tile_pool(name="xa", bufs=5) as xa,
                tc.tile_pool(name="stats", bufs=1) as stats,
                tc.tile_pool(name="psA", bufs=1, space="PSUM") as psA,
            ):
                wq_sb = wA.tile([128, NDC, LAT], bf16)
                d_ps = [
                    psA.tile([128, 512], f32, tag=f"d{lc}", name=f"d_ps{lc}")
                    for lc in range(NLC)
                ]
                for dc in range(NDC):
                    if dc % 2 == 0:
                        c2 = slice(dc * 128, (dc + 2) * 128)
                        xst2 = xa.tile(
                            [128, 2, 512],
                            bf16,
                            tag=f"xs{(dc // 2) % 4}",
                            bufs=1,
                            name="xst2",
                        )
                        nc.sync.dma_start(
                            out=xst2,
                            in_=xs[c2, :].rearrange("(dc p) n -> p dc n", p=128),
                        )
                        wdt2 = wdtp.tile(
                            [128, 2, 512], bf16, tag="wdt", bufs=4, name="wdt2"
                        )
                        nc.scalar.dma_start(
                            out=wdt2,
                            in_=wd[c2, :].rearrange("(dc p) n -> p dc n", p=128),
                        )
                    if dc == 8:
                        nc.sync.dma_start(
                            out=wq_sb,
                            in_=wq[:, :].rearrange("(dc p) n -> p dc n", p=128),
                        )
                    if dc == 12:
                        consts_dmas_early()
                    for lc in range(NLC):
                        nc.tensor.matmul(
                            d_ps[lc],
                            lhsT=wdt2[:, dc % 2, lc * 128 : (lc + 1) * 128],
                            rhs=xst2[:, dc % 2, :],
                            start=(dc == 0),
                            stop=(dc == NDC - 1),
                        )
                # LayerNorm stats over the 512 latent dims (4 tiles x 128
                # partitions). Sums and squares read PSUM directly to shorten
                # the latency chain into the collective. Pack [sum | sumsq]
                # side by side; one gpsimd cross-partition all-reduce
                # produces both, replicated.
                # LN stats via PE ones-matmuls: the sum over the 512
                # latent dims is a cross-partition reduction -- one matmul
                # against a ones column block reduces AND broadcasts, far
                # shorter than the copy/add/gpsimd-all-reduce chain (the
                # collective start time rides on this latency).
                dnb = [
                    stats.tile([128, 512], bf16, tag=f"dnb{i}", name=f"dnb{i}")
                    for i in range(4)
                ]
                for i in range(4):
                    if i % 2 == 0:
                        nc.scalar.copy(out=dnb[i], in_=d_ps[i])
                    else:
                        nc.vector.tensor_copy(dnb[i], d_ps[i])
                sqb = [
                    stats.tile([128, 512], bf16, tag=f"sqb{i}", name=f"sqb{i}")
                    for i in range(4)
                ]
                for i in range(4):
                    eng = nc.vector if i % 2 == 0 else nc.gpsimd
                    eng.tensor_mul(sqb[i], dnb[i], dnb[i])
                sum_ps = psA.tile([128, 512], f32, tag="d0", name="lnsum")
                ssq_ps = psA.tile([128, 512], f32, tag="d1", name="lnssq")
                for i in range(4):
                    nc.tensor.matmul(
                        sum_ps,
                        lhsT=ones_bf,
                        rhs=dnb[i],
                        start=(i == 0),
                        stop=(i == 3),
                    )
                for i in range(4):
                    nc.tensor.matmul(
                        ssq_ps,
                        lhsT=ones_bf,
                        rhs=sqb[i],
                        start=(i == 0),
                        stop=(i == 3),
                    )
                mu = stats.tile([128, 512], f32, tag="mu", name="mu")
                nc.vector.tensor_scalar_mul(mu, sum_ps, 1.0 / LAT)
                vtmp = stats.tile([128, 512], f32, tag="vt", name="vtmp")
                nc.vector.tensor_mul(vtmp, mu, mu)
                sd = stats.tile([128, 512], f32, tag="sd", name="sd")
                nc.vector.scalar_tensor_tensor(
                    out=sd,
                    in0=ssq_ps,
                    scalar=1.0 / LAT,
                    in1=vtmp,
                    op0=OP.mult,
                    op1=OP.subtract,
                )
                nc.scalar.activation(sd, sd, AF.Sqrt, bias=eps_sb)
                rstd = stats.tile([128, 512], f32, tag="rstd", name="rstd")
                nc.vector.reciprocal_approx_fast(rstd, sd)
                lat_stage = stats.tile(
                    [128, NLC, 512], bf16, tag="lstg", name="lat_stage"
                )
                for lc in range(NLC):
                    eng = nc.vector if lc < 2 else nc.gpsimd
                    t1 = stats.tile(
                        [128, 512], f32, tag=f"t1{lc % 2}", name="t1"
                    )
                    eng.tensor_sub(t1, dnb[lc], mu)
                    t2 = stats.tile(
                        [128, 512], f32, tag=f"t2{lc % 2}", name="t2"
                    )
                    eng.tensor_mul(t2, t1, rstd)
                    eng.tensor_scalar(
                        lat_stage[:, lc, :],
                        t2,
                        g_sb[:, lc : lc + 1],
                        b_sb[:, lc : lc + 1],
                        OP.mult,
                        OP.add,
                    )
                # AllGather the 4 query-tile slices of the new latent among
                # the 4 cores of this batch (the tensor-parallel group)
                latn_part = dramp.tile([128, NLC, 512], bf16)
                latn_all = dramp.tile([4, 128, NLC, 512], bf16)
                nc.gpsimd.dma_start(latn_part[:, :, :], lat_stage)
                nc.gpsimd.collective_compute(
                    "AllGather",
                    mybir.AluOpType.bypass,
                    replica_groups=[[0, 1, 2, 3], [4, 5, 6, 7]],
                    ins=[latn_part.opt()],
                    outs=[latn_all.opt()],
                )
                nc.gpsimd.dma_start(
                    latn_sb[:, :, :].rearrange("p lc (j n) -> p lc j n", j=4),
                    latn_all[:, :, :, :].rearrange("j p lc n -> p lc j n"),
                )
                # q projection
                for tt in range(NTT):
                    tsl = slice(tt * 512, (tt + 1) * 512)
                    q_ps = [
                        psA.tile([128, 512], f32, tag=f"q{qc}", name=f"q_ps{qc}")
                        for qc in range(HPC)
                    ]
                    for dc in range(NDC):
                        xt = xa.tile([128, 512], bf16, tag="xt", name="xt")
                        nc.sync.dma_start(
                            out=xt, in_=xT[dc * 128 : (dc + 1) * 128, tsl]
                        )
                        if tt >= 1 and dc in (0, 8):
                            consts_dmas_late((tt - 1) * 2 + (dc // 8))
                        for qc in range(HPC):
                            nc.tensor.matmul(
                                q_ps[qc],
                                lhsT=wq_sb[:, dc, qc * 128 : (qc + 1) * 128],
                                rhs=xt,
                                start=(dc == 0),
                                stop=(dc == NDC - 1),
                            )
                    for qc in range(HPC):
                        if qc % 2 == 0:
                            nc.vector.tensor_copy(qT_sb[:, qc, tsl], q_ps[qc])
                        else:
                            nc.scalar.copy(out=qT_sb[:, qc, tsl], in_=q_ps[qc])

            # wo prefetch overlaps phases B+C; pool stays open through D
            with tc.tile_pool(name="wD", bufs=1) as wD:
                wo_sb = wD.tile([128, HPC, D], bf16)
                for dt_ in range(D // 512):
                    nc.sync.dma_start(
                        out=wo_sb[:, :, dt_ * 512 : (dt_ + 1) * 512],
                        in_=wo[:, dt_ * 512 : (dt_ + 1) * 512].rearrange(
                            "(hc p) n -> p hc n", p=128
                        ),
                    )

                # ---- Phase B: k/v up-projection for all 4 heads
                with (
                    tc.tile_pool(name="psB", bufs=1, space="PSUM") as psB,
                ):
                    for g_ in range(PAST // 512):
                        ssl = slice(g_ * 512, (g_ + 1) * 512)

                        def latf(lc, _g=g_, _ssl=ssl):
                            return lp_sb[:, lc, _ssl]

                        for h in range(HPC):
                            k_ps = psB.tile(
                                [128, 512], f32, tag="kps", bufs=2, name="k_ps"
                            )
                            for lc in range(NLC):
                                nc.tensor.matmul(
                                    k_ps,
                                    lhsT=wk_sb[:, lc, h * 128 : (h + 1) * 128],
                                    rhs=latf(lc),
                                    start=(lc == 0),
                                    stop=(lc == NLC - 1),
                                )
                            if h % 2 == 0:
                                nc.scalar.copy(out=kT_sb[:, h, ssl], in_=k_ps)
                            else:
                                nc.vector.tensor_copy(kT_sb[:, h, ssl], k_ps)
                        for j4 in range(4):
                            # all 4 heads in one 512-wide moving dim
                            v_ps = psB.tile(
                                [128, 512], f32, tag="vps", bufs=2, name="v_ps"
                            )
                            for lc in range(NLC):
                                nc.tensor.matmul(
                                    v_ps,
                                    lhsT=latf(lc)[:, j4 * 128 : (j4 + 1) * 128],
                                    rhs=wv_sb[:, lc, :],
                                    start=(lc == 0),
                                    stop=(lc == NLC - 1),
                                )
                            if j4 % 2 == 0:
                                nc.vector.tensor_copy(
                                    vH_sb[:, g_ * 4 + j4, :], v_ps
                                )
                            else:
                                nc.scalar.copy(
                                    out=vH_sb[:, g_ * 4 + j4, :], in_=v_ps
                                )

                # ---- Phase C+D: attention per (query tile, head), with the
                # output projection for each query tile interleaved right
                # after its 4 heads finish. The o-proj PE work fills the gaps
                # while the scalar engine (exp) paces the attention stream.
                with (
                    tc.tile_pool(name="pp", bufs=1) as pp,
                    tc.tile_pool(name="ctmp", bufs=2) as ctmp,
                    tc.tile_pool(name="ost", bufs=4) as ost,
                    tc.tile_pool(name="psC", bufs=1, space="PSUM") as psC,
                ):
                    pidx = 0
                    ojobs = []

                    def emit_bnew(g_):
                        ssl = slice(g_ * 512, (g_ + 1) * 512)
                        nsl = slice((g_ - NTT) * 512, (g_ - NTT + 1) * 512)
                        for h in range(HPC):
                            k_ps = psC.tile(
                                [128, 512], f32, tag="ops", bufs=2, name="k_ps"
                            )
                            for lc in range(NLC):
                                nc.tensor.matmul(
                                    k_ps,
                                    lhsT=wk_sb[:, lc, h * 128 : (h + 1) * 128],
                                    rhs=latn_sb[:, lc, nsl],
                                    start=(lc == 0),
                                    stop=(lc == NLC - 1),
                                )
                            if h % 2 == 0:
                                nc.scalar.copy(out=kT_sb[:, h, ssl], in_=k_ps)
                            else:
                                nc.vector.tensor_copy(kT_sb[:, h, ssl], k_ps)
                        for j4 in range(4):
                            v_ps = psC.tile(
                                [128, 512], f32, tag="ops", bufs=2, name="v_ps"
                            )
                            for lc in range(NLC):
                                nc.tensor.matmul(
                                    v_ps,
                                    lhsT=latn_sb[:, lc, nsl][
                                        :, j4 * 128 : (j4 + 1) * 128
                                    ],
                                    rhs=wv_sb[:, lc, :],
                                    start=(lc == 0),
                                    stop=(lc == NLC - 1),
                                )
                            if j4 % 2 == 0:
                                nc.vector.tensor_copy(vH_sb[:, g_ * 4 + j4, :], v_ps)
                            else:
                                nc.scalar.copy(out=vH_sb[:, g_ * 4 + j4, :], in_=v_ps)

                    def emit_ojobs(n, act_ok=False):
                        for _ in range(min(n, len(ojobs))):
                            dt_, tc_ = ojobs.pop(0)
                            o_ps = psC.tile(
                                [128, 512], f32, tag="ops", bufs=2, name="o_ps"
                            )
                            for hc in range(HPC):
                                nc.tensor.matmul(
                                    o_ps,
                                    lhsT=ao_sb[:, hc, tc_ * 128 : (tc_ + 1) * 128],
                                    rhs=wo_sb[:, hc, dt_ * 512 : (dt_ + 1) * 512],
                                    start=(hc == 0),
                                    stop=(hc == HPC - 1),
                                )
                            o_sb = ost.tile([128, 512], f32, tag="osb", name="o_sb")
                            if act_ok and tc_ % 2 == 0:
                                nc.scalar.copy(out=o_sb, in_=o_ps)
                            else:
                                nc.vector.tensor_copy(o_sb, o_ps)
                            nc.sync.dma_start(
                                out=o[
                                    tc_ * 128 : (tc_ + 1) * 128,
                                    dt_ * 512 : (dt_ + 1) * 512,
                                ],
                                in_=o_sb,
                            )

                    for tt in range(NTT):
                        tsl = slice(tt * 512, (tt + 1) * 512)
                        for h in range(HPC):
                            hsl = slice(h * 128, (h + 1) * 128)
                            # the very first stage runs all its past pairs
                            # BEFORE the deferred g4 k/v group, hiding the
                            # latent AllGather latency; its diagonal blocks
                            # (which need g4) then close the psum groups.
                            diags_last = tt == 0 and h == 0
                            # full key blocks go in PAIRS (two score matmuls
                            # into one 2-bank psum tile, ONE exp over 1024
                            # cols). First pair opens the psum groups
                            # full-width; the 4 diagonal blocks (trimmed to
                            # their causally-visible columns) go next.
                            fulls = list(range(NPB)) + [
                                NPB + jn for jn in range(4 * tt)
                            ]
                            prs = [
                                (fulls[2 * i], fulls[2 * i + 1])
                                for i in range(len(fulls) // 2)
                            ]
                            diags = [(NPB + 4 * tt + i, i * 128) for i in range(4)]
                            ad = psC.tile([128, 1024], f32, tag="ad", bufs=1, name="ad")
                            attn_ps = ad[:, 0:512]
                            den_ps = ad[:, 512:1024]
                            pdiag = pp.tile([128, 512], bf16, tag="pd", name="pdiag")
                            state = {"p8q": None, "dr": None}

                            def flush_dr(_ad=ad, _state=None):
                                st = _state if _state is not None else state
                                if st["dr"] is not None:
                                    q8, first = st["dr"]
                                    nc.tensor.matmul(
                                        _ad[:, 512:1024],
                                        lhsT=ones8,
                                        rhs=q8,
                                        start=first,
                                        stop=False,
                                        perf_mode=DR,
                                    )
                                    st["dr"] = None

                            def do_pair(fi, ja, jbb, _h=h, _tsl=tsl, _hsl=hsl,
                                        _ad=ad, _npr=len(prs), _state=state,
                                        _pv_stop=(not diags_last)):
                                global _PIDX
                                sp2 = psC.tile(
                                    [128, 1024], f32, tag="sps", bufs=2, name="sp2"
                                )
                                for i, jj in enumerate((ja, jbb)):
                                    nc.tensor.matmul(
                                        sp2[:, i * 512 : (i + 1) * 512],
                                        lhsT=kT_sb[:, _h, jj * 128 : (jj + 1) * 128],
                                        rhs=qT_sb[:, _h, _tsl],
                                        start=True,
                                        stop=True,
                                    )
                                flush_dr(_state=_state)
                                p2 = pp.tile(
                                    [128, 2, 512], bf16, tag=f"p{_PIDX % 4}",
                                    name="p2",
                                )
                                _PIDX += 1
                                nc.scalar.activation(
                                    p2[:, :, :], sp2, AF.Exp, scale=SCALE
                                )
                                for i, jj in enumerate((ja, jbb)):
                                    nc.tensor.matmul(
                                        _ad[:, 0:512],
                                        lhsT=vH_sb[:, jj, _hsl],
                                        rhs=p2[:, i, :],
                                        start=(fi == 0 and i == 0),
                                        stop=(
                                            _pv_stop
                                            and fi == _npr - 1
                                            and i == 1
                                        ),
                                    )
                                if fi % 2 == 0:
                                    _state["p8q"] = pp.tile(
                                        [128, 2, 512],
                                        f8,
                                        tag=f"p8q{(fi // 2) % 2}",
                                        name="p8q",
                                    )
                                # alternate Pool/DVE for the fp8 pair-sums
                                eng = nc.gpsimd if fi % 2 == 0 else nc.vector
                                eng.tensor_add(
                                    _state["p8q"][:, fi % 2, :], p2[:, 0, :],
                                    p2[:, 1, :],
                                )
                                if fi % 2 == 1:
                                    _state["dr"] = (_state["p8q"], fi == 1)

                            do_pair(0, *prs[0])
                            if diags_last:
                                for fi in range(1, len(prs)):
                                    do_pair(fi, *prs[fi])
                                emit_bnew(4)
                            ndiag = len(diags)
                            for di, (jb, rr) in enumerate(diags):
                                s_ps = psC.tile(
                                    [128, 1024], f32, tag="sps", bufs=2, name="s_ps"
                                )
                                nc.tensor.matmul(
                                    s_ps[:, rr:512],
                                    lhsT=kT_sb[:, h, jb * 128 : (jb + 1) * 128],
                                    rhs=qT_sb[:, h, tt * 512 + rr : (tt + 1) * 512],
                                    start=True,
                                    stop=True,
                                )
                                p_sb = pp.tile(
                                    [128, 512], bf16, tag=f"pdg{rr // 128 % 2}",
                                    name="p_sb",
                                )
                                nc.scalar.activation(
                                    p_sb[:, rr:], s_ps[:, rr:512], AF.Exp,
                                    scale=SCALE,
                                )
                                nc.vector.tensor_mul(
                                    p_sb[:, rr : rr + 128],
                                    p_sb[:, rr : rr + 128],
                                    mask_sb[:, 384:512],
                                )
                                nc.tensor.matmul(
                                    attn_ps[:, rr:],
                                    lhsT=vH_sb[:, jb, hsl],
                                    rhs=p_sb[:, rr:],
                                    start=False,
                                    stop=(diags_last and di == ndiag - 1),
                                )
                                if rr == 0:
                                    nc.vector.tensor_copy(pdiag, p_sb)
                                else:
                                    nc.vector.tensor_add(
                                        pdiag[:, rr:], pdiag[:, rr:], p_sb[:, rr:]
                                    )
                            emit_ojobs(2)
                            if not diags_last:
                                for fi in range(1, len(prs)):
                                    do_pair(fi, *prs[fi])
                            flush_dr()
                            # diagonal blocks' denominator contribution
                            nc.tensor.matmul(
                                den_ps,
                                lhsT=ones_bf,
                                rhs=pdiag,
                                start=False,
                                stop=True,
                            )
                            rec = ctmp.tile([128, 512], f32, tag="rec", name="rec")
                            nc.vector.reciprocal_approx_fast(rec, den_ps)
                            nc.vector.tensor_mul(ao_sb[:, h, tsl], attn_ps, rec)
                            emit_ojobs(3)
                            if tt == 0 and h < 3:
                                emit_bnew(5 + h)
                        # queue this query tile's output-projection tiles;
                        # they are drained 5-at-a-time after later attention
                        # stages so their PE work absorbs exp (ACT) pacing.
                        ojobs.extend(
                            (dt_, tc_)
                            for dt_ in range(D // 512)
                            for tc_ in range(tt * 4, tt * 4 + 4)
                        )
                    emit_ojobs(len(ojobs), act_ok=True)

    nc.compile()
    return nc


def _get_nc():
    if "nc" not in _CACHE:
        _CACHE["nc"] = _build()
    return _CACHE["nc"]


def _make_mask():
    # B[jj, c] = 1.0 iff c >= jj + 384 for c < 512; cols [512:640) are all
    # ones. Sliced at [384:512] it is the causal staircase of one diagonal
    # 128-key block: query col c sees key jj iff c >= jj.
    import ml_dtypes

    jj = np.arange(128)[:, None]
    cc = np.arange(640)[None, :]
    return (cc >= jj + 384).astype(ml_dtypes.bfloat16)


def _in_maps(x, latent_prev, Wq, Wdown, Wk_up, Wv_up, ln_g, ln_b, Wo):
    import ml_dtypes

    bf = lambda a: np.ascontiguousarray(
        np.asarray(a, dtype=np.float32).astype(ml_dtypes.bfloat16)
    )
    f = lambda a: np.ascontiguousarray(np.asarray(a, dtype=np.float32))
    mask = _make_mask()
    maps = []
    for bi in range(2):
        xTb = bf(np.asarray(x)[bi].T)
        lpTb = bf(np.asarray(latent_prev)[bi].T)
        for hg in range(4):
            sl = slice(hg * 512, (hg + 1) * 512)
            maps.append(
                {
                    "xT": xTb,
                    "xs": np.ascontiguousarray(xTb[:, sl]),
                    "lpT": lpTb,
                    "wq": bf(np.asarray(Wq)[:, sl]),
                    "wd": bf(Wdown),
                    "wk": bf(np.asarray(Wk_up)[:, sl]),
                    "wv": bf(np.asarray(Wv_up)[:, sl]),
                    "wo": bf(np.asarray(Wo)[sl, :]),
                    "g": f(ln_g),
                    "bb": f(ln_b),
                    "mask": mask,
                }
            )
    return maps


def run(trace=False, **inputs):
    from concourse.bass_utils import run_bass_kernel_spmd

    nc = _get_nc()
    maps = _in_maps(**inputs)
    res = run_bass_kernel_spmd(nc, maps, core_ids=list(range(8)), trace=trace)
    outs = [res.results[c]["o"] for c in range(8)]
    out = np.stack(
        [
            outs[0] + outs[1] + outs[2] + outs[3],
            outs[4] + outs[5] + outs[6] + outs[7],
        ],
        axis=0,
    ).astype(np.float32)
    return out, res


def kernel(**inputs):
    out, _ = run(trace=False, **inputs)
    return out
